# revision 34
# baseline (speedup 1.0000x reference)
"""Trainium2 Bass kernel for DecomposableAttention (B=512, L=256, V=50000, E=300, H=200).

Data-parallel over batch across 8 cores (64 batches/core).  All matmuls bf16
(1 cycle/row), fp32 PSUM.  Per batch:

- indirect-DMA gather of bf16 embedding rows (table converted host-side) with a
  ones/mask column appended for the softmax denominators
- PE transposes to the E-on-partitions layout; both sentences packed side by
  side in one [128, 2560] x tile so every MLP matmul/activation runs once with
  a 512-wide free dim
- attend MLP -> scores (one direction) -> exp without the max stabilizer
  (scores bounded ~11; the stabilizer cancels exactly after normalization);
  the reverse-direction exp'd scores come from a PE transpose of u0
- attention sums in [token, 301] layout: the denominator lands in column 300 of
  the same matmul, so 1/(den+eps) is a [128,1] per-partition op and the
  normalization + output masking fuse into one tensor_scalar; the result is
  PE-transposed into the compare layout
- compare MLP over 5 packed 128-row K-chunks; masked column-sum fused via a
  K=1 logmask matmul into PSUM + the activation engine's accum_out

The per-batch work is software-pipelined with a 1-batch skew: H1(b+1)
(gather -> eT transposes -> attend -> scores -> exp) is emitted before H2(b)
(u1 transpose -> attention sums -> normalize -> compare), so the in-order PE
queue always has independent matmuls to run during the serial softmax chain's
engine handoffs.  Hidden dim is zero-padded 200 -> 256 so every weight chunk
is a full 128 columns.

PSUM (8 banks x 2KB): psT 2 (bf16 transpose staging), psMM 3 (MLP), psS 1
(scores), psA 2 (attention sums).  Accumulation groups never interleave within
a bank (hardware breaks interleaved groups) and pool buffers never straddle
bank boundaries.
"""
import sys

if '/opt/trn_rl_repo' not in sys.path:
    sys.path.insert(0, '/opt/trn_rl_repo')

import numpy as np

B, L, VOCAB, EMBED, HIDDEN = 512, 256, 50000, 300, 200
NCORES = 8
BC = B // NCORES  # batches per core
P = 128
E1 = EMBED + 1    # emb cols + ones/mask column

_prog_cache = {}

# K-chunk layout of the 600-dim compare input [e (300); beta (300)].
# Chunk 2 mixes the e tail (44 rows) + 20 zero rows + beta[0:64] at partition
# offset 64; W1c is host-packed to match.
XCHUNK = [128, 128, 128, 128, 108]
BCHUNK = [(0, 64, 64), (64, 192, 0), (192, 300, 0)]  # beta col ranges + partition offset


def build_program(nb=BC, ndev=NCORES, dbg=False):
    import concourse.bass as bass
    import concourse.bacc as bacc
    import concourse.tile as tile
    import concourse.mybir as mybir
    from concourse.masks import make_identity

    F32 = mybir.dt.float32
    BF = mybir.dt.bfloat16
    I32 = mybir.dt.int32
    ALU = mybir.AluOpType
    ACTF = mybir.ActivationFunctionType
    EK = [(0, 128), (128, 256), (256, 300)]   # E contraction chunks
    HM = [(0, 128), (128, 256)]               # H chunks, zero-padded to 2x128
    HP = 256                                  # padded hidden dim

    nc = bacc.Bacc("TRN2", num_devices=ndev)

    emb_d = nc.dram_tensor("emb", [VOCAB, EMBED], BF, kind="ExternalInput")
    s1_d = nc.dram_tensor("s1", [nb, L], I32, kind="ExternalInput")
    s2_d = nc.dram_tensor("s2", [nb, L], I32, kind="ExternalInput")
    len1_d = nc.dram_tensor("len1", [nb, 1], I32, kind="ExternalInput")
    len2_d = nc.dram_tensor("len2", [nb, 1], I32, kind="ExternalInput")
    W1a_d = nc.dram_tensor("W1a", [EMBED, 256], BF, kind="ExternalInput")
    W2a_d = nc.dram_tensor("W2a", [256, 256], BF, kind="ExternalInput")
    W1c_d = nc.dram_tensor("W1c", [640, 256], BF, kind="ExternalInput")   # host-packed 5x128
    W2c_d = nc.dram_tensor("W2c", [256, 256], BF, kind="ExternalInput")
    W1g_d = nc.dram_tensor("W1g", [512, 256], BF, kind="ExternalInput")
    W2g_d = nc.dram_tensor("W2g", [256, 2], BF, kind="ExternalInput")
    b1a_d = nc.dram_tensor("b1a", [256, 1], F32, kind="ExternalInput")
    b2a_d = nc.dram_tensor("b2a", [256, 1], F32, kind="ExternalInput")
    b1c_d = nc.dram_tensor("b1c", [256, 1], F32, kind="ExternalInput")
    b2c_d = nc.dram_tensor("b2c", [256, 1], F32, kind="ExternalInput")
    b1g_d = nc.dram_tensor("b1g", [256, 1], F32, kind="ExternalInput")
    b2g_d = nc.dram_tensor("b2g", [2, 1], F32, kind="ExternalInput")
    out_d = nc.dram_tensor("out", [nb, 2], F32, kind="ExternalOutput")
    if dbg:
        dbg_d = {
            'XT': nc.dram_tensor("dbg_XT", [P, 10 * L], BF, kind="ExternalOutput"),
            'u0': nc.dram_tensor("dbg_u0", [P, 2 * L], BF, kind="ExternalOutput"),
            'u1': nc.dram_tensor("dbg_u1", [P, 2 * L], BF, kind="ExternalOutput"),
            'hT0': nc.dram_tensor("dbg_hT0", [P, 2 * L], BF, kind="ExternalOutput"),
            'v00': nc.dram_tensor("dbg_v00", [P, nb], F32, kind="ExternalOutput"),
        }

    with tile.TileContext(nc) as tc:
        import contextlib
        ctx = contextlib.ExitStack()
        with ctx:
            const = ctx.enter_context(tc.tile_pool(name="const", bufs=1))
            psT = ctx.enter_context(tc.tile_pool(name="psT", bufs=2, space="PSUM"))
            psMM = ctx.enter_context(tc.tile_pool(name="psMM", bufs=3, space="PSUM"))
            psS = ctx.enter_context(tc.tile_pool(name="psS", bufs=1, space="PSUM"))
            psA = ctx.enter_context(tc.tile_pool(name="psA", bufs=2, space="PSUM"))
            gat = ctx.enter_context(tc.tile_pool(name="gat", bufs=3))
            xtp = ctx.enter_context(tc.tile_pool(name="xtp", bufs=2))
            hp = ctx.enter_context(tc.tile_pool(name="hp", bufs=3))
            sm = ctx.enter_context(tc.tile_pool(name="sm", bufs=3))

            def ps_pair(pool, nm):
                return pool.tile([P, 512], F32, name=nm, tag=pool.name)

            def ps_tr(nm):
                # full 2KB so every pool buffer is PSUM-bank aligned
                return psT.tile([P, 1024], BF, name=nm, tag="psT")

            # ---------------- constants ----------------
            ident = const.tile([P, P], F32)
            make_identity(nc, ident[:])
            ident_b = const.tile([P, P], BF)
            nc.vector.tensor_copy(ident_b[:], ident[:])
            ones_row_b = const.tile([1, P], BF)
            nc.vector.memset(ones_row_b[:], 1.0)

            # weights (bf16, direct DMA)
            W1a_t = [const.tile([k1 - k0, HP], BF, name=f"W1a{i}", tag=f"W1a{i}")
                     for i, (k0, k1) in enumerate(EK)]
            for i, (k0, k1) in enumerate(EK):
                nc.sync.dma_start(W1a_t[i][:], W1a_d[k0:k1, :])
            W2a_t = [const.tile([m1 - m0, HP], BF, name=f"W2a{i}", tag=f"W2a{i}")
                     for i, (m0, m1) in enumerate(HM)]
            for i, (m0, m1) in enumerate(HM):
                nc.sync.dma_start(W2a_t[i][:], W2a_d[m0:m1, :])
            W1c_t = [const.tile([k, HP], BF, name=f"W1c{i}", tag=f"W1c{i}")
                     for i, k in enumerate(XCHUNK)]
            for i in range(5):
                nc.sync.dma_start(W1c_t[i][:], W1c_d[i * 128:i * 128 + XCHUNK[i], :])
            W2c_t = [const.tile([m1 - m0, HP], BF, name=f"W2c{i}", tag=f"W2c{i}")
                     for i, (m0, m1) in enumerate(HM)]
            for i, (m0, m1) in enumerate(HM):
                nc.sync.dma_start(W2c_t[i][:], W2c_d[m0:m1, :])
            GK = [(0, 128), (128, 256), (256, 384), (384, 512)]
            W1g_t = [const.tile([k1 - k0, HP], BF, name=f"W1g{i}", tag=f"W1g{i}")
                     for i, (k0, k1) in enumerate(GK)]
            for i, (k0, k1) in enumerate(GK):
                nc.sync.dma_start(W1g_t[i][:], W1g_d[k0:k1, :])
            W2g_t = [const.tile([m1 - m0, 2], BF, name=f"W2g{i}", tag=f"W2g{i}")
                     for i, (m0, m1) in enumerate(HM)]
            for i, (m0, m1) in enumerate(HM):
                nc.sync.dma_start(W2g_t[i][:], W2g_d[m0:m1, :])

            def bias2(d, nm):
                t = [const.tile([m1 - m0, 1], F32, name=f"b{nm}{i}", tag=f"b{nm}{i}")
                     for i, (m0, m1) in enumerate(HM)]
                for i, (m0, m1) in enumerate(HM):
                    nc.sync.dma_start(t[i][:], d[m0:m1, :])
                return t
            b1a_t, b2a_t = bias2(b1a_d, "1a"), bias2(b2a_d, "2a")
            b1c_t, b2c_t = bias2(b1c_d, "1c"), bias2(b2c_d, "2c")
            b1g_t = bias2(b1g_d, "1g")
            b2g_t = const.tile([2, 1], F32)
            nc.sync.dma_start(b2g_t[:], b2g_d[:])

            # masks / lengths
            len_t = []
            len_f = []
            for s, ld in ((0, len1_d), (1, len2_d)):
                lt = const.tile([nb, 1], I32, name=f"len{s}", tag=f"len{s}")
                nc.sync.dma_start(lt[:], ld[:])
                len_t.append(lt)
                lf = const.tile([1, nb], I32, name=f"lenf{s}", tag=f"lenf{s}")
                nc.sync.dma_start(lf[:], ld[:].rearrange("n one -> one n"))
                len_f.append(lf)
            iota_t = const.tile([nb, L], I32)
            nc.gpsimd.iota(iota_t[:], pattern=[[1, L]], base=0, channel_multiplier=0)
            iota2 = const.tile([1, 2 * L], I32)   # 0..255 twice
            nc.gpsimd.iota(iota2[:], pattern=[[0, 2], [1, L]], base=0, channel_multiplier=0)

            lmT = []   # transposed logmask1 [128, nb] f32, 2 chunks (exp bias)
            mTb = []   # transposed 0/1 masks [128, nb] bf16, 2 chunks per sentence
            mTf = []   # same masks in f32 (tensor_scalar scalar2 operand)
            for s in range(2):
                m = const.tile([nb, L], F32, name=f"mask{s}", tag=f"mask{s}")
                nc.vector.tensor_tensor(m[:], iota_t[:], len_t[s][:].to_broadcast([nb, L]), op=ALU.is_lt)
                lts, mts, mfs = [], [], []
                for c in range(2):
                    if s == 0:
                        lm = const.tile([nb, L], F32, name="lm0", tag="lm0")
                        nc.vector.tensor_scalar(lm[:], m[:], 1.0, 30000.0,
                                                op0=ALU.subtract, op1=ALU.mult)
                        tp = ps_pair(psS, "setup_tp")
                        nc.tensor.transpose(tp[:, 0:nb], lm[:, c * P:(c + 1) * P], ident[:nb, :nb])
                        lt = const.tile([P, nb], F32, name=f"lmT{s}{c}", tag=f"lmT{s}{c}")
                        nc.vector.tensor_copy(lt[:], tp[:, 0:nb])
                        lts.append(lt)
                    tp2 = ps_pair(psS, "setup_tp")
                    nc.tensor.transpose(tp2[:, 0:nb], m[:, c * P:(c + 1) * P], ident[:nb, :nb])
                    mt = const.tile([P, nb], BF, name=f"mTb{s}{c}", tag=f"mTb{s}{c}")
                    nc.vector.tensor_copy(mt[:], tp2[:, 0:nb])
                    mts.append(mt)
                    mf = const.tile([P, nb], F32, name=f"mTf{s}{c}", tag=f"mTf{s}{c}")
                    nc.vector.tensor_copy(mf[:], tp2[:, 0:nb])
                    mfs.append(mf)
                lmT.append(lts)
                mTb.append(mts)
                mTf.append(mfs)

            # token indices, transposed to [128, nb] int32 per chunk
            sT = []
            for s, sd in ((0, s1_d), (1, s2_d)):
                st = const.tile([nb, L], I32, name=f"s{s}", tag=f"s{s}")
                nc.sync.dma_start(st[:], sd[:])
                sf = const.tile([nb, L], F32, name=f"sf{s}", tag=f"sf{s}")
                nc.vector.tensor_copy(sf[:], st[:])
                chunks = []
                for c in range(2):
                    tp = ps_pair(psS, "setup_tp")
                    nc.tensor.transpose(tp[:, 0:nb], sf[:, c * P:(c + 1) * P], ident[:nb, :nb])
                    tf = const.tile([P, nb], F32, name=f"sTf{s}{c}", tag=f"sTf{s}{c}")
                    nc.vector.tensor_copy(tf[:], tp[:, 0:nb])
                    ti = const.tile([P, nb], I32, name=f"sTi{s}{c}", tag=f"sTi{s}{c}")
                    nc.vector.tensor_copy(ti[:], tf[:])
                    chunks.append(ti)
                sT.append(chunks)

            # v accumulators [(128|72), nb] per H-chunk per sentence
            v_all = [[const.tile([m1 - m0, nb], F32, name=f"v{s}{m}", tag=f"v{s}{m}")
                      for m, (m0, m1) in enumerate(HM)] for s in range(2)]

            # ---------------- per-batch loop (1-batch software pipeline) ----
            # H1(b): gather -> eT transposes -> attend MLP -> scores -> exp.
            # H2(b): u1 transpose -> attention sums -> normalize -> compare.
            # Emission order H1(b+1) before H2(b) keeps independent PE work
            # between the engine handoffs of the serial softmax chain.
            def emit_h1(b):
                st = {'b': b}
                lens2 = sm.tile([1, 2 * L], I32, name="lens2", tag="lens2")
                for s in range(2):
                    nc.vector.tensor_copy(lens2[:, s * L:(s + 1) * L],
                                          len_f[s][:, b:b + 1].to_broadcast([1, L]))
                mrow2 = sm.tile([1, 2 * L], F32, name="mrow2", tag="mrow2")
                nc.vector.tensor_tensor(mrow2[:], iota2[:], lens2[:], op=ALU.is_lt)
                lmrow2 = sm.tile([1, 2 * L], BF, name="lmrow2", tag="lmrow2")
                nc.vector.tensor_scalar(lmrow2[:], mrow2[:], 1.0, 30000.0,
                                        op0=ALU.subtract, op1=ALU.mult)
                st['lmrow2'] = lmrow2

                eR = [[], []]
                for s in range(2):
                    for c in range(2):
                        er = gat.tile([P, E1], BF, name=f"eR{s}{c}", tag=f"eR{s}{c}")
                        nc.gpsimd.indirect_dma_start(
                            out=er[:, 0:EMBED], out_offset=None, in_=emb_d[:],
                            in_offset=bass.IndirectOffsetOnAxis(ap=sT[s][c][:, b:b + 1], axis=0),
                        )
                        if b < 3:
                            nc.vector.memset(er[:, EMBED:E1], 1.0)
                        eR[s].append(er)
                e2m = []
                for c in range(2):
                    em = gat.tile([P, E1], BF, name=f"e2m{c}", tag=f"e2m{c}")
                    nc.gpsimd.tensor_tensor(em[:], eR[1][c][:],
                                            mTb[1][c][:, b:b + 1].to_broadcast([P, E1]), op=ALU.mult)
                    e2m.append(em)
                st['eR'] = eR
                st['e2m'] = e2m

                XT = xtp.tile([P, 10 * L], BF, name="XT", tag="XT")
                for s in range(2):
                    if b < 2:
                        nc.vector.memset(XT[0:64, 4 * L + s * L:4 * L + (s + 1) * L], 0.0)
                    tp = ps_tr(f"eTtp{s}")
                    for k, (k0, k1) in enumerate(EK):
                        for c in range(2):
                            nc.tensor.transpose(tp[:k1 - k0, k * L + c * P:k * L + (c + 1) * P],
                                                eR[s][c][:, k0:k1], ident_b[:])
                    for k in range(2):
                        nc.any.tensor_copy(XT[:, 2 * k * L + s * L:2 * k * L + (s + 1) * L],
                                           tp[:, k * L:(k + 1) * L])
                    nc.any.tensor_copy(XT[0:44, 4 * L + s * L:4 * L + (s + 1) * L],
                                       tp[0:44, 2 * L:3 * L])
                st['XT'] = XT

                ha = []
                for m, (m0, m1) in enumerate(HM):
                    pa = ps_pair(psMM, "pa")
                    for k, (k0, k1) in enumerate(EK):
                        nc.tensor.matmul(pa[:m1 - m0, :], W1a_t[k][:, m0:m1],
                                         XT[0:k1 - k0, 2 * k * L:2 * (k + 1) * L],
                                         start=(k == 0), stop=(k == 2))
                    h = hp.tile([m1 - m0, 2 * L], BF, name=f"ha{m}", tag=f"ha{m}")
                    nc.scalar.activation(h[:], pa[:m1 - m0, :], ACTF.Relu,
                                         bias=b1a_t[m][:], scale=1.0)
                    ha.append(h)
                hT = []
                for m, (m0, m1) in enumerate(HM):
                    pb = ps_pair(psMM, "pb")
                    for k2 in range(2):
                        nc.tensor.matmul(pb[:m1 - m0, :], W2a_t[k2][:, m0:m1], ha[k2][:],
                                         start=(k2 == 0), stop=(k2 == 1))
                    h = hp.tile([m1 - m0, 2 * L], BF, name=f"hT{m}", tag=f"hT{m}")
                    nc.scalar.activation(h[:], pb[:m1 - m0, :], ACTF.Relu,
                                         bias=b2a_t[m][:], scale=1.0)
                    hT.append(h)
                st['hT'] = hT

                ep = ps_pair(psS, "score")
                for ic in range(2):
                    for m in range(2):
                        nc.tensor.matmul(ep[:, ic * L:(ic + 1) * L],
                                         hT[m][:, ic * P:(ic + 1) * P], hT[m][:, L:2 * L],
                                         start=(m == 0), stop=(m == 1))
                u0 = sm.tile([P, 2 * L], BF, name="u0", tag="u0")
                for ic in range(2):
                    nc.scalar.activation(u0[:, ic * L:(ic + 1) * L], ep[:, ic * L:(ic + 1) * L],
                                         ACTF.Exp, bias=lmT[0][ic][:, b:b + 1], scale=1.0)
                st['u0'] = u0
                return st

            def emit_h2(st):
                b = st['b']
                eR, e2m, XT, u0 = st['eR'], st['e2m'], st['XT'], st['u0']
                # phase A: u1 transpose + all attention-sum matmuls; the DVE
                # normalize chain per q tile runs while the PE keeps going
                tpu = ps_tr("u1tp")
                for jc in range(2):
                    for ic in range(2):
                        nc.tensor.transpose(tpu[:, jc * L + ic * P:jc * L + (ic + 1) * P],
                                            u0[:, ic * L + jc * P:ic * L + (jc + 1) * P], ident_b[:])
                u1 = sm.tile([P, 2 * L], BF, name="u1", tag="u1")
                nc.any.tensor_copy(u1[:], tpu[:, 0:2 * L])

                nrms = {}
                for d in range(2):
                    u_t = u0 if d == 0 else u1
                    rhs = eR[0] if d == 0 else e2m
                    for t_ in range(2):
                        q = ps_pair(psA, f"q{d}{t_}")
                        for c in range(2):
                            nc.tensor.matmul(q[:, 0:E1],
                                             u_t[:, c * L + t_ * P:c * L + (t_ + 1) * P],
                                             rhs[c][:, 0:E1], start=(c == 0), stop=(c == 1))
                        rcp = sm.tile([P, 1], F32, name=f"rcp{d}{t_}", tag=f"rcp{d}{t_}")
                        if d == 0:
                            # den_A >= exp(min score) > 0: no epsilon needed
                            nc.vector.reciprocal(rcp[:], q[:, EMBED:E1])
                        else:
                            dcol = sm.tile([P, 1], F32, name=f"dc{t_}", tag=f"dc{t_}")
                            nc.vector.tensor_scalar(dcol[:], q[:, EMBED:E1], 1e-20, None,
                                                    op0=ALU.add)
                            nc.vector.reciprocal(rcp[:], dcol[:])
                        nrm = sm.tile([P, EMBED], BF, name=f"nrm{d}{t_}", tag=f"nrm{d}{t_}")
                        nc.vector.tensor_scalar(nrm[:], q[:, 0:EMBED], rcp[:],
                                                mTf[1 - d][t_][:, b:b + 1],
                                                op0=ALU.mult, op1=ALU.mult)
                        nrms[(d, t_)] = nrm

                # phase B: transpose normalized sums into the compare layout
                for d in range(2):
                    s = 1 - d
                    tt = ps_tr(f"bt{d}")
                    for t_ in range(2):
                        nrm = nrms[(d, t_)]
                        for t, (c0, c1, poff) in enumerate(BCHUNK):
                            nc.tensor.transpose(
                                tt[poff:poff + (c1 - c0), t * L + t_ * P:t * L + (t_ + 1) * P],
                                nrm[:, c0:c1], ident_b[:])
                    nc.any.tensor_copy(XT[64:128, 4 * L + s * L:4 * L + (s + 1) * L],
                                       tt[64:128, 0:L])
                    nc.any.tensor_copy(XT[:, 6 * L + s * L:6 * L + (s + 1) * L],
                                       tt[:, L:2 * L])
                    nc.any.tensor_copy(XT[0:108, 8 * L + s * L:8 * L + (s + 1) * L],
                                       tt[0:108, 2 * L:3 * L])

                # phase C: compare MLP (N=512)
                r1 = []
                for m, (m0, m1) in enumerate(HM):
                    pc = ps_pair(psMM, "pc")
                    for k in range(5):
                        nc.tensor.matmul(pc[:m1 - m0, :], W1c_t[k][:, m0:m1],
                                         XT[0:XCHUNK[k], 2 * k * L:2 * (k + 1) * L],
                                         start=(k == 0), stop=(k == 4))
                    r = hp.tile([m1 - m0, 2 * L], BF, name=f"r1{m}", tag=f"r1{m}")
                    nc.scalar.activation(r[:], pc[:m1 - m0, :], ACTF.Relu,
                                         bias=b1c_t[m][:], scale=1.0)
                    r1.append(r)
                for m, (m0, m1) in enumerate(HM):
                    pd = ps_pair(psMM, "pd")
                    for k2 in range(2):
                        nc.tensor.matmul(pd[:m1 - m0, :], W2c_t[k2][:, m0:m1], r1[k2][:],
                                         start=(k2 == 0), stop=False)
                    nc.tensor.matmul(pd[:m1 - m0, :], ones_row_b[:, 0:m1 - m0],
                                     st['lmrow2'][:], start=False, stop=True)
                    for s in range(2):
                        scr = hp.tile([m1 - m0, L], BF, name=f"scr{s}{m}", tag=f"scr{s}{m}")
                        nc.scalar.activation(scr[:], pd[:m1 - m0, s * L:(s + 1) * L],
                                             ACTF.Relu, bias=b2c_t[m][:],
                                             scale=1.0, accum_out=v_all[s][m][:, b:b + 1])

                if dbg and b == 0:
                    nc.sync.dma_start(dbg_d['XT'][:], XT[:])
                    nc.sync.dma_start(dbg_d['u0'][:], u0[:])
                    nc.sync.dma_start(dbg_d['u1'][:], u1[:])
                    nc.sync.dma_start(dbg_d['hT0'][:], st['hT'][0][:])

            prev = emit_h1(0)
            for b in range(1, nb):
                cur = emit_h1(b)
                emit_h2(prev)
                prev = cur
            emit_h2(prev)

            # ---------------- aggregate ----------------
            if dbg:
                nc.sync.dma_start(dbg_d['v00'][:], v_all[0][0][:])
            vb = []
            for s in range(2):
                for m, (m0, m1) in enumerate(HM):
                    t = const.tile([m1 - m0, nb], BF, name=f"vb{s}{m}", tag=f"vb{s}{m}")
                    nc.vector.tensor_copy(t[:], v_all[s][m][:])
                    vb.append(t)
            g1 = []
            gp = ps_pair(psMM, "pa")
            for m, (m0, m1) in enumerate(HM):
                for k in range(4):
                    nc.tensor.matmul(gp[:m1 - m0, m * nb:(m + 1) * nb],
                                     W1g_t[k][:, m0:m1], vb[k][:],
                                     start=(k == 0), stop=(k == 3))
            for m, (m0, m1) in enumerate(HM):
                g = const.tile([m1 - m0, nb], BF, name=f"g1{m}", tag=f"g1{m}")
                nc.scalar.activation(g[:], gp[:m1 - m0, m * nb:(m + 1) * nb],
                                     ACTF.Relu, bias=b1g_t[m][:], scale=1.0)
                g1.append(g)
            op = ps_pair(psMM, "pb")
            for k2 in range(2):
                nc.tensor.matmul(op[0:2, 0:nb], W2g_t[k2][:], g1[k2][:],
                                 start=(k2 == 0), stop=(k2 == 1))
            osb = const.tile([2, nb], F32, name="osb", tag="osb")
            nc.scalar.activation(osb[:], op[0:2, 0:nb], ACTF.Identity, bias=b2g_t[:], scale=1.0)
            nc.sync.dma_start(out_d[:].rearrange("b o -> o b"), osb[:])

    nc.compile()
    return nc


def _shard_inputs(inputs, nb=BC, ncores=NCORES):
    import ml_dtypes
    bf16 = ml_dtypes.bfloat16
    f = np.ascontiguousarray

    emb_bf = f(inputs['emb'].astype(bf16))
    # Hidden dim zero-padded 200 -> 256 so every weight chunk is a full 128
    # columns (enables the PE fast weight load).  W1c additionally packed into
    # 5 chunks of 128 rows: [0:256] e-rows, chunk2 = 44 e-tail rows + 20 zero
    # rows + 64 beta rows, then beta rows 64:192, 192:300.
    HPad = 256
    W1c = inputs['W1c'].astype(np.float32)
    W1c_p = np.zeros((640, HPad), np.float32)
    W1c_p[0:256, 0:HIDDEN] = W1c[0:256]
    W1c_p[256:300, 0:HIDDEN] = W1c[256:300]
    W1c_p[320:384, 0:HIDDEN] = W1c[300:364]
    W1c_p[384:512, 0:HIDDEN] = W1c[364:492]
    W1c_p[512:620, 0:HIDDEN] = W1c[492:600]
    W1a_p = np.zeros((EMBED, HPad), np.float32)
    W1a_p[:, 0:HIDDEN] = inputs['W1a']
    W2a_p = np.zeros((HPad, HPad), np.float32)
    W2a_p[0:HIDDEN, 0:HIDDEN] = inputs['W2a']
    W2c_p = np.zeros((HPad, HPad), np.float32)
    W2c_p[0:HIDDEN, 0:HIDDEN] = inputs['W2c']
    # v layout is [s0m0 (128) | s0m1 (72+56 pad) | s1m0 | s1m1]
    W1g = inputs['W1g'].astype(np.float32)
    W1g_p = np.zeros((512, HPad), np.float32)
    W1g_p[0:128, 0:HIDDEN] = W1g[0:128]
    W1g_p[128:200, 0:HIDDEN] = W1g[128:200]
    W1g_p[256:384, 0:HIDDEN] = W1g[200:328]
    W1g_p[384:456, 0:HIDDEN] = W1g[328:400]
    W2g_p = np.zeros((HPad, 2), np.float32)
    W2g_p[0:HIDDEN] = inputs['W2g']
    wb = {'W1a': f(W1a_p.astype(bf16)), 'W2a': f(W2a_p.astype(bf16)),
          'W1c': f(W1c_p.astype(bf16)), 'W2c': f(W2c_p.astype(bf16)),
          'W1g': f(W1g_p.astype(bf16)), 'W2g': f(W2g_p.astype(bf16))}

    def bpad(x):
        p = np.zeros((256, 1), np.float32)
        p[0:HIDDEN, 0] = np.asarray(x).ravel()
        return p

    maps = []
    for c in range(ncores):
        sl = slice(c * nb, (c + 1) * nb)
        maps.append(dict(
            emb=emb_bf,
            s1=f(inputs['s1'][sl].astype(np.int32)),
            s2=f(inputs['s2'][sl].astype(np.int32)),
            len1=f(inputs['len1'][sl].reshape(nb, 1).astype(np.int32)),
            len2=f(inputs['len2'][sl].reshape(nb, 1).astype(np.int32)),
            b1a=bpad(inputs['b1a']), b2a=bpad(inputs['b2a']),
            b1c=bpad(inputs['b1c']), b2c=bpad(inputs['b2c']),
            b1g=bpad(inputs['b1g']),
            b2g=f(inputs['b2g'].reshape(-1, 1).astype(np.float32)),
            **wb,
        ))
    return maps


def kernel(**inputs):
    from concourse.bass_utils import run_bass_kernel_spmd
    if 'prog' not in _prog_cache:
        _prog_cache['prog'] = build_program(BC)
    nc = _prog_cache['prog']
    in_maps = _shard_inputs(inputs)
    res = run_bass_kernel_spmd(nc, in_maps, core_ids=list(range(NCORES)))
    out = np.concatenate([res.results[c]["out"] for c in range(NCORES)], axis=0)
    return out.astype(np.float32)
